# revision 1
# baseline (speedup 1.0000x reference)
"""Trainium2 Bass kernel for nn_EquivariantMatrix (group conv over Z16 x Z16).

Math: out[b,f,h] = sum_{i,s} kernel[f,i,s] * x[b,i,h (-) s] + bias[f]
(2D circular convolution over the 16x16 translation group; the reference's
536MB expanded-kernel tensor is never materialized).

Sharding: data-parallel over batch, 2 batches per core on 8 cores.

Per-core device plan (fp32 data, float32r matmul mode):
  - xe[t][p=(s2off*32+i), g1pad, (h2,bl)], g1pad in [0,32) doubled:
    value x[b0+bl, i, g1pad%16, (h2-(4t+s2off))%16]  (host-prepared, 2MB)
  - ktt[t][p=(s2off*32+i), col=(pp*128+s1off*64+f)] = kernel[f,i,2pp+s1off,4t+s2off]
  - one psum tile (128,512) accumulates, for t in 0..3, pp in 0..7, a single
    N=512 matmul whose rhs window offset (16-2pp)%16 into the doubled g1pad
    axis aligns even s1=2pp with the output h1; odd s1=2pp+1 lands rotated by
    one h1. First matmul carries start=True (it covers the whole tile).
  - the raw psum is bounced to SBUF (DMA cannot read PSUM) and shipped out;
    the odd-half h1-rotation, the cross-half add and the bias add happen on
    the host during assembly.
  - PE warm-up: full-array K=128 dummy matmuls into a scratch psum bank during
    the DMA prologue so HAM un-throttles before the stream.
"""

import numpy as np

L1 = L2 = 16
S = 256
I = 32
F = 64
B = 16
NCORES = 8
BPC = 2  # batches per core
N_WARMUP = 20


def _np_f32(a):
    return np.ascontiguousarray(np.asarray(a), dtype=np.float32)


_cache = {}


def _build_nc():
    from concourse import bacc
    import concourse.tile as tile
    import concourse.mybir as mybir

    f32 = mybir.dt.float32
    f32r = mybir.dt.float32r

    nc = bacc.Bacc(None, target_bir_lowering=False, debug=False)
    comb_d = nc.dram_tensor("comb", (4, 128, 1536), f32r, kind="ExternalInput")
    out_d = nc.dram_tensor("out", (2, 128, 512), f32, kind="ExternalOutput")

    with tile.TileContext(nc) as tc:
        with (
            tc.tile_pool(name="data", bufs=1) as pool,
            tc.tile_pool(name="ps", bufs=1, space="PSUM") as pspool,
        ):
            # comb[t] cols: [0:1024) kt, [1024:1536) xe, [1536:2048) xe pad
            comb = [pool.tile([128, 2048], f32r, name=f"comb{t}", tag=f"comb{t}")
                    for t in range(4)]
            wu = pool.tile([128, 256], f32r, tag="wu")
            psum_a = pspool.tile([128, 512], f32, tag="psum_a")
            psum_b = pspool.tile([128, 512], f32, tag="psum_b")
            scratch = pspool.tile([128, 512], f32, tag="scratch")

            # warm-up operand from a memset (no DMA dependency -> PE
            # activity starts during the instruction-load head); f32r has no
            # memset encoding, so zero it through a uint32 view
            nc.gpsimd.memset(wu[:].bitcast(mybir.dt.uint32), 0)

            # ---- prologue DMAs, issue split across the idle scalar
            # sequencer and sync so the 4 issues serialize 2-deep, not 4 ----
            for t in range(4):
                eng = nc.sync if t % 2 == 0 else nc.scalar
                eng.dma_start(comb[t][:, 0:1536], comb_d[t])

            # ---- PE warm-up: full-array dummies into a scratch bank ----
            for w in range(N_WARMUP):
                nc.tensor.matmul(scratch[:, 0:256], wu[:, 0:128], wu[:],
                                 start=True, stop=True,
                                 skip_group_check=True)

            # ---- duplicate xe into its padded half (fast contiguous DVE) ----
            for t in range(4):
                nc.vector.tensor_copy(comb[t][:, 1536:2048],
                                      comb[t][:, 1024:1536])

            # ---- main accumulation: 32 matmuls, all N=512 contiguous.
            # Phases t0-1 accumulate into psum_a, t2-3 into psum_b, so A's
            # drain copy + out-DMA hide under the second half of the stream;
            # the host sums the two raw partials. ----
            out_a = pool.tile([128, 512], f32, tag="out_a")
            out_b = pool.tile([128, 512], f32, tag="out_b")
            for t in range(4):
                ps = psum_a if t < 2 else psum_b
                for pp in range(8):
                    goff = (16 - 2 * pp) % 16  # pp=0 -> unpadded half
                    lhsT = comb[t][:, pp * 128:(pp + 1) * 128]
                    rhs = comb[t][:, 1024 + goff * 32:1024 + goff * 32 + 512]
                    nc.tensor.matmul(ps[:], lhsT, rhs,
                                     start=(t in (0, 2) and pp == 0),
                                     stop=(t in (1, 3) and pp == 7),
                                     skip_group_check=True)
                if t == 1:
                    nc.vector.tensor_copy(out_a[:], psum_a[:])
                    nc.sync.dma_start(out_d[0], out_a[:])
            nc.vector.tensor_copy(out_b[:], psum_b[:])
            nc.sync.dma_start(out_d[1], out_b[:])

    nc.finalize()
    return nc


def _host_prep_kt(kern):
    # ktt[t, p=(s2off*32+i), pp*128 + s1off*64 + f] = kern[f, i, 2pp+s1off, 4t+s2off]
    k4 = kern.reshape(F, I, 8, 2, 4, 4)          # f, i, pp, s1off, t, s2off
    kt = k4.transpose(4, 5, 1, 2, 3, 0)          # t, s2off, i, pp, s1off, f
    return np.ascontiguousarray(kt.reshape(4, 128, 1024), dtype=np.float32)


def _host_prep_xe(xc):
    # xe[t, s2off*32+i, g1*32 + h2*2 + bl] = xc[bl, i, g1, (h2-(4t+s2off))%16]
    x4 = xc.reshape(BPC, I, L1, L2)
    xe = np.empty((4, 128, 512), np.float32)
    for t in range(4):
        for s2off in range(4):
            s2 = 4 * t + s2off
            sh = np.roll(x4, s2, axis=3).transpose(1, 2, 3, 0)  # i, g1, h2, bl
            xe[t, s2off * 32:(s2off + 1) * 32] = sh.reshape(I, 512)
    return xe


def _make_in_maps(x, kern, bias):
    kt = _host_prep_kt(kern)
    maps = []
    for c in range(NCORES):
        xe = _host_prep_xe(x[BPC * c:BPC * (c + 1)])
        combv = np.concatenate([kt, xe], axis=2)   # (4, 128, 1536)
        maps.append({"comb": np.ascontiguousarray(combv)})
    return maps


def _assemble(results, bias):
    out = np.empty((B, F, S), np.float32)
    for c in range(NCORES):
        ph = results[c]["out"]                       # (2, 128, 512) partials
        p = ph[0] + ph[1]
        o = np.empty((F, 512), np.float32)
        # even-s1 half + odd-s1 half rotated by +1 in h1 (32-col blocks)
        o[:, 32:512] = p[0:64, 32:512] + p[64:128, 0:480]
        o[:, 0:32] = p[0:64, 0:32] + p[64:128, 480:512]
        o += bias[:, None]
        o = o.reshape(F, L1, L2, BPC).transpose(3, 0, 1, 2)
        out[BPC * c:BPC * (c + 1)] = o.reshape(BPC, F, S)
    return out


def kernel(x, kernel, bias, product_table):
    from concourse.bass_utils import run_bass_kernel_spmd

    if _cache.get("nc") is None:
        _cache["nc"] = _build_nc()

    bias = _np_f32(bias)
    in_maps = _make_in_maps(_np_f32(x), _np_f32(kernel), bias)
    # the device occasionally reports a transient NRT_EXEC_UNIT_UNRECOVERABLE
    # on the first touch; a retry has always succeeded
    last_err = None
    for _ in range(3):
        try:
            res = run_bass_kernel_spmd(_cache["nc"], in_maps,
                                       list(range(NCORES)))
            return _assemble(res.results, bias)
        except Exception as e:  # noqa: BLE001
            last_err = e
    raise last_err



# revision 3
# speedup vs baseline: 1.6473x; 1.6473x over previous
"""Trainium2 Bass kernel for nn_EquivariantMatrix (group conv over Z16 x Z16).

Math: out[b,f,h1,h2] = sum_{i,s1,s2} kernel[f,i,s1,s2] * x[b,i,(h1-s1)%16,(h2-s2)%16]
(2D circular convolution; the reference's 536MB expanded kernel is never
materialized).

Algorithm: rfft-16 along the second lattice axis (g2) on the host turns the
s2-convolution into 9 independent per-frequency-bin products. Sharding is
tensor-parallel over the bins: cores 0-6 each own one complex bin (1..7),
core 7 owns the two real bins (0 and 8) packed as a block-diagonal "complex"
pair — every core runs the identical program on different data.

Per-core device work: 8 accumulating matmuls, K=128 = (2 s1-steps x 32
in-features x re/im), M=128 = (re/im out x 64 features), N=256 = (h1 major x
batch minor). The s1-shift is realized as a column-window offset into an
h1-doubled rhs buffer whose second partition half is pre-shifted by one h1
step, so a single window serves both s1 values of a K-block exactly.

Host does only the cheap length-16 DFT transforms (~15 MFLOP total) and data
layout; the device performs the full (i, s1)-contraction (252 MFLOP).
"""

import numpy as np
import ml_dtypes

L1 = L2 = 16
S = 256
I = 32
F = 64
B = 16
NCORES = 8
N_WARMUP = 6

_cache = {}


def _build_nc():
    from concourse import bacc
    import concourse.tile as tile
    import concourse.mybir as mybir

    f32 = mybir.dt.float32
    bf16 = mybir.dt.bfloat16

    nc = bacc.Bacc(None, target_bir_lowering=False, debug=False)
    # comb cols: [0:1024) W (pp-major, 8 blocks of 128), [1024:1536) XB
    comb_d = nc.dram_tensor("comb", (128, 1536), bf16, kind="ExternalInput")
    out_d = nc.dram_tensor("out", (128, 256), f32, kind="ExternalOutput")

    with tile.TileContext(nc) as tc:
        with (
            tc.tile_pool(name="data", bufs=1) as pool,
            tc.tile_pool(name="ps", bufs=1, space="PSUM") as pspool,
        ):
            comb = pool.tile([128, 1536], bf16, tag="comb")
            wu = pool.tile([128, 256], bf16, tag="wu")
            outs = pool.tile([128, 256], f32, tag="outs")
            psum = pspool.tile([128, 256], f32, tag="psum")
            scratch = pspool.tile([128, 256], f32, tag="scratch")

            # warm-up operand from a memset (no DMA dependency -> PE busy
            # during the DMA prologue); bf16 has no memset encoding, so
            # zero through a uint32 view
            nc.gpsimd.memset(wu[:].bitcast(mybir.dt.uint32), 0)

            # prologue DMAs: XB first (every matmul needs it), then W halves
            nc.sync.dma_start(comb[:, 1024:1536], comb_d[:, 1024:1536])
            nc.scalar.dma_start(comb[:, 0:512], comb_d[:, 0:512])
            nc.sync.dma_start(comb[:, 512:1024], comb_d[:, 512:1024])

            for _ in range(N_WARMUP):
                nc.tensor.matmul(scratch[:], wu[:, 0:128], wu[:],
                                 start=True, stop=True,
                                 skip_group_check=True)

            # main accumulation: 8 matmuls K=128 N=256 into one psum tile
            for pp in range(8):
                o = (16 - 2 * pp) % 16
                lhsT = comb[:, pp * 128:(pp + 1) * 128]
                rhs = comb[:, 1024 + o * 16:1024 + o * 16 + 256]
                nc.tensor.matmul(psum[:], lhsT, rhs,
                                 start=(pp == 0), stop=(pp == 7),
                                 skip_group_check=True)

            nc.vector.tensor_copy(outs[:], psum[:])
            nc.sync.dma_start(out_d[:], outs[:])

    nc.finalize()
    return nc


def _build_core_data(xh, kh, core):
    """W[8,128,128], XB[128,512] (float64) for one core.

    cores 0-6: complex bin core+1; core 7: real bins (0, 8) block-diagonal.
    """
    if core < 7:
        w = core + 1
        xp = np.stack([xh[..., w].real, xh[..., w].imag], axis=-1)
        kr, ki = kh[..., w].real, kh[..., w].imag
        Wfull = np.empty((F, I, L1, 2, 2))  # f, i, s1, pin, pout
        Wfull[..., 0, 0] = kr
        Wfull[..., 1, 0] = -ki
        Wfull[..., 0, 1] = ki
        Wfull[..., 1, 1] = kr
    else:
        xp = np.stack([xh[..., 0].real, xh[..., 8].real], axis=-1)
        Wfull = np.zeros((F, I, L1, 2, 2))
        Wfull[..., 0, 0] = kh[..., 0].real
        Wfull[..., 1, 1] = kh[..., 8].real

    # W[pp, (s1off, i, pin), (pout, f)]
    Wt = Wfull.transpose(2, 1, 3, 4, 0)          # s1, i, pin, pout, f
    W = np.ascontiguousarray(Wt).reshape(8, 2, I, 2, 2, F).reshape(8, 128, 128)

    # XB[(s1off, i, pin), j*16 + b] = xp[b, i, (j - s1off) % 16, pin]
    base = xp.transpose(1, 3, 2, 0)              # i, pin, g1, b
    t3 = np.concatenate([base, base, base], axis=2)  # g1 tripled
    xb0 = t3[:, :, 0:32]                         # (j) % 16
    xb1 = t3[:, :, 15:47]                        # (j - 1) % 16
    XB = np.stack([xb0, xb1], axis=0)            # s1off, i, pin, j, b
    return W, XB.reshape(2 * I * 2, 32 * B).reshape(128, 512)


def _make_in_maps(x, kern):
    x4 = np.asarray(x, np.float32).reshape(B, I, L1, L2)
    k4 = np.asarray(kern, np.float32).reshape(F, I, L1, L2)
    xh = np.fft.rfft(x4, axis=3)                 # (B, I, 16, 9)
    kh = np.fft.rfft(k4, axis=3)                 # (F, I, 16, 9)
    maps = []
    for c in range(NCORES):
        W, XB = _build_core_data(xh, kh, c)
        comb = np.empty((128, 1536), dtype=ml_dtypes.bfloat16)
        comb[:, 0:1024] = np.concatenate(list(W), axis=1).astype(ml_dtypes.bfloat16)
        comb[:, 1024:1536] = XB.astype(ml_dtypes.bfloat16)
        maps.append({"comb": comb})
    return maps


def _assemble(results, bias):
    out_hat = np.empty((B, F, L1, 9), np.complex128)
    for c in range(NCORES):
        ps = np.asarray(results[c]["out"], np.float64)   # [(pout,f), (h1,b)]
        lo = ps[:64].reshape(F, L1, B).transpose(2, 0, 1)
        hi = ps[64:].reshape(F, L1, B).transpose(2, 0, 1)
        if c < 7:
            out_hat[..., c + 1] = lo + 1j * hi
        else:
            out_hat[..., 0] = lo
            out_hat[..., 8] = hi
    out = np.fft.irfft(out_hat, n=L2, axis=3).reshape(B, F, S)
    out = out + np.asarray(bias, np.float64)[None, :, None]
    return np.ascontiguousarray(out, dtype=np.float32)


def kernel(x, kernel, bias, product_table):
    from concourse.bass_utils import run_bass_kernel_spmd

    if _cache.get("nc") is None:
        _cache["nc"] = _build_nc()

    in_maps = _make_in_maps(x, kernel)
    # the device occasionally reports a transient NRT_EXEC_UNIT_UNRECOVERABLE
    # on the first touch; a retry has always succeeded
    last_err = None
    for _ in range(3):
        try:
            res = run_bass_kernel_spmd(_cache["nc"], in_maps,
                                       list(range(NCORES)))
            return _assemble(res.results, bias)
        except Exception as e:  # noqa: BLE001
            last_err = e
    raise last_err
